# revision 7
# baseline (speedup 1.0000x reference)
"""ChannelAttention Trainium2 Bass kernel.

Full (unsharded) inputs -> full output. Data-parallel over batch B=8 across
the 8 NeuronCores (one batch element per core, SPMD program, no collectives).

Per-core math (N=4096 tokens, C=512 channels):
    qkv = x @ Wqkv + bqkv ; q,k,v = relu(split(qkv))
    scores = (q^T k) / sqrt(C)           # [C, C] contraction over tokens
    attn = softmax(scores, -1) * adj
    y = v @ attn ; out = y @ Wo + bo

Matmuls run in float32r (fp32 storage, ~1e-4 rel err, full PE rate).
"""

import sys

sys.path.insert(0, "/opt/trn_rl_repo")

from contextlib import ExitStack

import numpy as np

import concourse.bass as bass
import concourse.mybir as mybir
import concourse.tile as tile
from concourse import bacc
from concourse.bass import ds, ts
from concourse.bass_utils import run_bass_kernel_spmd
from concourse.masks import make_identity

# Problem shape (hardcoded per contract).
B, N, C = 8, 4096, 512
P = 128
CC = C // P            # channel chunks (4)
NT = N // P            # token tiles (32)
TPS = 4                # token tiles per slab
NS = NT // TPS         # slabs (8)
SLAB = TPS * P         # tokens per slab (512)

F32 = mybir.dt.float32
F32R = mybir.dt.float32r

_CACHE = {}


def build(reps: int = 1):
    nc = bacc.Bacc("TRN2", target_bir_lowering=False, debug=False, num_devices=8)

    x = nc.dram_tensor("x", [N, C], F32, kind="ExternalInput").ap()
    adj = nc.dram_tensor("adj", [C, C], F32, kind="ExternalInput").ap()
    wqkv = nc.dram_tensor("Wqkv", [C, 3 * C], F32, kind="ExternalInput").ap()
    bqkv = nc.dram_tensor("bqkv", [3 * C], F32, kind="ExternalInput").ap()
    wo = nc.dram_tensor("Wo", [C, C], F32, kind="ExternalInput").ap()
    bo = nc.dram_tensor("bo", [C], F32, kind="ExternalInput").ap()
    out = nc.dram_tensor("out", [N, C], F32, kind="ExternalOutput").ap()

    s = 1.0 / float(np.sqrt(C))

    with tile.TileContext(nc) as tc, ExitStack() as ctx:
        const = ctx.enter_context(tc.tile_pool(name="const", bufs=1))

        # ---- constants -------------------------------------------------
        with tc.tile_pool(name="stage", bufs=1) as stage:
            wqkv_f = stage.tile([P, CC, 3 * C], F32, tag="stage_wqkv")
            nc.sync.dma_start(wqkv_f[:], wqkv.rearrange("(o p) d -> p o d", p=P))
            wqkv_r = const.tile([P, CC, 3 * C], F32R)
            nc.vector.tensor_copy(wqkv_r[:], wqkv_f[:])

            wo_f = stage.tile([P, CC, C], F32, tag="stage_wo")
            nc.sync.dma_start(wo_f[:], wo.rearrange("(o p) d -> p o d", p=P))
            wo_r = const.tile([P, CC, C], F32R)
            nc.vector.tensor_copy(wo_r[:], wo_f[:])

            brow_f = stage.tile([1, 3 * C], F32, tag="stage_b")
            nc.sync.dma_start(brow_f[:], bqkv[None, :])
            brow_r = const.tile([1, 3 * C], F32R)
            nc.vector.tensor_copy(brow_r[:], brow_f[:])

            borow_f = stage.tile([1, C], F32, tag="stage_bo")
            nc.sync.dma_start(borow_f[:], bo[None, :])
            borow_r = const.tile([1, C], F32R)
            nc.vector.tensor_copy(borow_r[:], borow_f[:])

            ones_f = stage.tile([1, P], F32, tag="stage_ones")
            nc.gpsimd.memset(ones_f[:], 1.0)
            ones_r = const.tile([1, P], F32R)
            nc.vector.tensor_copy(ones_r[:], ones_f[:])

        # v-bias, per-partition layout [p, chunk]
        bv = const.tile([P, CC], F32)
        nc.sync.dma_start(bv[:], bqkv[2 * C :].rearrange("(o p) -> p o", p=P))

        ident = const.tile([P, P], F32)
        make_identity(nc, ident[:])

        adj_sb = const.tile([P, CC, C], F32)
        nc.sync.dma_start(adj_sb[:], adj.rearrange("(o p) d -> p o d", p=P))

        vt_sb = const.tile([P, CC, N], F32R)      # v^T, channel-major
        attn_sb = const.tile([P, CC, C], F32R)    # gated softmax rows

        # ---- pass 1: qkv projection + channel scores -------------------
        scores_pool = ctx.enter_context(
            tc.tile_pool(name="scores", bufs=1, space="PSUM")
        )
        scores_ps = [
            scores_pool.tile([P, C], F32, tag=f"scores{o}", name=f"scores{o}")
            for o in range(CC)
        ]

        rep_ctx = tc.For_i(0, reps, 1) if reps > 1 else None
        if rep_ctx is not None:
            ctx.enter_context(rep_ctx)

        with (
            tc.tile_pool(name="tp_ps", bufs=2, space="PSUM") as tp_ps,
            tc.tile_pool(name="proj_ps", bufs=2, space="PSUM") as proj_ps,
            tc.tile_pool(name="xin", bufs=3) as xin,
            tc.tile_pool(name="xtp", bufs=2) as xtp,
            tc.tile_pool(name="qk", bufs=3) as qk,
        ):
            for sl in range(NS):
                xt_slab = xtp.tile([P, CC, SLAB], F32R, tag="xT")
                for tt in range(TPS):
                    t = sl * TPS + tt
                    x_t = xin.tile([P, C], F32, tag="x")
                    nc.sync.dma_start(x_t[:], x[ts(t, P), :])

                    # transpose 128x512 -> xT chunks via PE
                    pst = tp_ps.tile([P, C], F32, tag="tp")
                    for o in range(CC):
                        nc.tensor.transpose(pst[:, ts(o, P)], x_t[:, ts(o, P)], ident[:])
                    nc.vector.tensor_copy(
                        xt_slab[:, :, ts(tt, P)],
                        pst[:].rearrange("p (o n) -> p o n", o=CC),
                    )

                    # q = relu(x @ Wq + bq)   (token-major)
                    q_ps = proj_ps.tile([P, C], F32, tag="proj")
                    for o in range(CC):
                        nc.tensor.matmul(
                            q_ps[:],
                            xt_slab[:, o, ts(tt, P)],
                            wqkv_r[:, o, 0:C],
                            start=(o == 0),
                            stop=False,
                        )
                    nc.tensor.matmul(
                        q_ps[:], ones_r[:], brow_r[:, 0:C], start=False, stop=True
                    )
                    q_sb = qk.tile([P, C], F32R, tag="qk")
                    nc.scalar.activation(
                        q_sb[:], q_ps[:], mybir.ActivationFunctionType.Relu
                    )

                    # k = relu(x @ Wk + bk)
                    k_ps = proj_ps.tile([P, C], F32, tag="proj")
                    for o in range(CC):
                        nc.tensor.matmul(
                            k_ps[:],
                            xt_slab[:, o, ts(tt, P)],
                            wqkv_r[:, o, C : 2 * C],
                            start=(o == 0),
                            stop=False,
                        )
                    nc.tensor.matmul(
                        k_ps[:], ones_r[:], brow_r[:, C : 2 * C], start=False, stop=True
                    )
                    k_sb = qk.tile([P, C], F32R, tag="qk")
                    nc.vector.tensor_scalar_max(k_sb[:], k_ps[:], 0.0)

                    # scores[o] += q[:, o-chunk]^T @ k
                    for o in range(CC):
                        nc.tensor.matmul(
                            scores_ps[o][:],
                            q_sb[:, ts(o, P)],
                            k_sb[:],
                            start=(t == 0),
                            stop=(t == NT - 1),
                        )

                # vT[d, n] = relu(Wv^T x^T + bv)  (channel-major, kept in SBUF)
                for d in range(CC):
                    v_ps = proj_ps.tile([P, C], F32, tag="proj")
                    for o in range(CC):
                        nc.tensor.matmul(
                            v_ps[:, :SLAB],
                            wqkv_r[:, o, ds(2 * C + d * P, P)],
                            xt_slab[:, o, :],
                            start=(o == 0),
                            stop=(o == CC - 1),
                        )
                    nc.scalar.activation(
                        vt_sb[:, d, ts(sl, SLAB)],
                        v_ps[:, :SLAB],
                        mybir.ActivationFunctionType.Relu,
                        bias=bv[:, d : d + 1],
                    )

            # ---- softmax + adjacency gate ------------------------------
            with tc.tile_pool(name="smx", bufs=8) as smx:
                for o in range(CC):
                    smax = smx.tile([P, 1], F32, tag="smax")
                    nc.vector.reduce_max(
                        smax[:], scores_ps[o][:], axis=mybir.AxisListType.X
                    )
                    nbias = smx.tile([P, 1], F32, tag="nbias")
                    nc.vector.tensor_scalar_mul(nbias[:], smax[:], -s)
                    ssum = smx.tile([P, 1], F32, tag="ssum")
                    attn_e = smx.tile([P, C], F32, tag="attn_e")
                    nc.scalar.activation(
                        attn_e[:],
                        scores_ps[o][:],
                        mybir.ActivationFunctionType.Exp,
                        bias=nbias[:],
                        scale=s,
                        accum_out=ssum[:],
                    )
                    rsum = smx.tile([P, 1], F32, tag="rsum")
                    nc.vector.reciprocal(rsum[:], ssum[:])
                    attn_r = smx.tile([P, C], F32, tag="attn_r")
                    nc.vector.tensor_scalar_mul(attn_r[:], attn_e[:], rsum[:])
                    nc.vector.tensor_mul(
                        attn_sb[:, o, :], attn_r[:], adj_sb[:, o, :]
                    )

        # ---- pass 2: y = v @ attn ; out = y @ Wo + bo ------------------
        with (
            tc.tile_pool(name="y_ps", bufs=4, space="PSUM") as y_ps_pool,
            tc.tile_pool(name="yt", bufs=2) as ytp,
            tc.tile_pool(name="outp", bufs=3) as outp,
        ):
            for sl in range(NS):
                yt_slab = ytp.tile([P, CC, SLAB], F32R, tag="yT")
                for d in range(CC):
                    y_ps = y_ps_pool.tile([P, C], F32, tag="y")
                    for o in range(CC):
                        nc.tensor.matmul(
                            y_ps[:, :SLAB],
                            attn_sb[:, o, ts(d, P)],
                            vt_sb[:, o, ts(sl, SLAB)],
                            start=(o == 0),
                            stop=(o == CC - 1),
                        )
                    if d % 2 == 0:
                        nc.scalar.copy(yt_slab[:, d, :], y_ps[:, :SLAB])
                    else:
                        nc.vector.tensor_copy(yt_slab[:, d, :], y_ps[:, :SLAB])

                for tt in range(TPS):
                    t = sl * TPS + tt
                    o_ps = y_ps_pool.tile([P, C], F32, tag="y")
                    for d in range(CC):
                        nc.tensor.matmul(
                            o_ps[:],
                            yt_slab[:, d, ts(tt, P)],
                            wo_r[:, d, :],
                            start=(d == 0),
                            stop=False,
                        )
                    nc.tensor.matmul(
                        o_ps[:], ones_r[:], borow_r[:], start=False, stop=True
                    )
                    out_sb = outp.tile([P, C], F32, tag="out")
                    if tt % 2 == 0:
                        nc.scalar.copy(out_sb[:], o_ps[:])
                    else:
                        nc.vector.tensor_copy(out_sb[:], o_ps[:])
                    nc.sync.dma_start(out[ts(t, P), :], out_sb[:])

    nc.compile()
    return nc


def _get_nc(reps: int = 1):
    key = ("nc", reps)
    if key not in _CACHE:
        _CACHE[key] = build(reps)
    return _CACHE[key]


def _run(inputs, trace=False, reps: int = 1):
    nc = _get_nc(reps)
    x = np.ascontiguousarray(np.asarray(inputs["x"], dtype=np.float32))
    adj = np.ascontiguousarray(np.asarray(inputs["adj"], dtype=np.float32))
    wqkv = np.ascontiguousarray(np.asarray(inputs["Wqkv"], dtype=np.float32))
    bqkv = np.ascontiguousarray(np.asarray(inputs["bqkv"], dtype=np.float32))
    wo = np.ascontiguousarray(np.asarray(inputs["Wo"], dtype=np.float32))
    bo = np.ascontiguousarray(np.asarray(inputs["bo"], dtype=np.float32))

    in_maps = [
        {
            "x": x[b],
            "adj": adj[b],
            "Wqkv": wqkv,
            "bqkv": bqkv,
            "Wo": wo,
            "bo": bo,
        }
        for b in range(B)
    ]
    res = run_bass_kernel_spmd(
        nc, in_maps, core_ids=list(range(B)), trace=trace
    )
    outp = np.stack([res.results[b]["out"] for b in range(B)], axis=0)
    return outp.astype(np.float32), res


def kernel(**inputs) -> np.ndarray:
    out, _ = _run(inputs, trace=False)
    return out
